# revision 8
# baseline (speedup 1.0000x reference)
"""Cross-attention (q-norm variant) Trainium2 Bass kernel, v2.

Sharding: batch (2) x row-quarters (4) -> 8 cores, data-parallel over the
query sequence. Each core handles 1408 query rows (5376 padded to 5632 per
batch) of ONE batch, with that batch's context replicated. No collectives.

Key idea vs v1: the host pre-transposes x and context, so the kernel never
runs a single PE transpose -- every tensor-engine instruction is an
accumulating bf16 matmul. All attention tensors live in "transposed"
layouts:

  phase A: kT_h = wkv_k_h^T @ ctxT (per head) ; v = (ctxT chunks)^T @ wkv_v
  phase B per 512-row group, software-pipelined over heads h:
    qT_h   = wq_h^T @ xT_g                  (12 acc MMs -> PSUM fp32)
    qsq    = Square(qT_h)      [ACT]        -> SBUF bf16
    ssq    = ones^T @ qsq                   ([1,512] PSUM = sum_d q^2)
    rs     = Exp(-0.5*Ln(ssq + HD*eps))     [ACT, one table set]
    rsB    = partition_broadcast(rs)        [GPSIMD]
    qTn_h  = qT_h * rsB        [DVE]        -> SBUF bf16 (RMS-normed q^T)
    sT_jb  = kT_h,jb^T @ qTn_h              (4 MMs, scores transposed)
    eT_jb  = Exp(sT_jb)        [ACT]        -> SBUF bf16
    sums   = ones^T @ eT (acc 4)            ([1,512] = softmax denom)
    oT_h   = v_h,jb^T @ eT (acc 4)          (unnormalized out^T)
    rc     = reciprocal_approx_fast(sums)   [DVE]
    rcB    = partition_broadcast(rc)        [GPSIMD]
    oTn_h  = oT_h * rcB        [DVE]        -> SBUF bf16
  then per 128-row block: out = concat_h(oTn_h)^T @ wo chunks; DMA out.

RMS-norm folds the 1/sqrt(HD) attention scale (rs = (ssq + HD*eps)^-1/2);
q_norm_scale is folded into the k-half of wkv on the host. Biases are
structurally zero and dropped. Only ACT functions from the
natural_log_exp_and_others table set are used (Exp/Ln/Square): one
ACT_TABLE_LOAD for the whole kernel.
"""

import os
import sys
import numpy as np

for _p in ("/opt/trn_rl_repo",):
    if _p not in sys.path:
        sys.path.insert(0, _p)

import ml_dtypes
import concourse.bass as bass
import concourse.tile as tile
from concourse import bacc, mybir
from concourse import bass_utils
from concourse import library_config

F32 = mybir.dt.float32
BF16 = mybir.dt.bfloat16
EXP = mybir.ActivationFunctionType.Exp
LN = mybir.ActivationFunctionType.Ln
SQUARE = mybir.ActivationFunctionType.Square

B, N, D, M, H, HD = 2, 5376, 1536, 512, 12, 128
EPS = 1e-6
NCORES = 8
CPB = 4            # cores per batch
RPC = 1408         # padded rows per core  (4*1408 = 5632 >= 5376)
NBLK = RPC // 128  # 11
DC = D // 128      # 12 contraction chunks
JB = M // 128      # 4 context row blocks
GROUPS = [(0, 4), (4, 4), (8, 3)]   # (start block, #blocks)

TRACE = False

_cache = {}


def _build():
    nc = bacc.Bacc(
        "TRN2", target_bir_lowering=False, debug=False, num_devices=NCORES
    )
    xT_d = nc.dram_tensor("xT", [D, RPC], BF16, kind="ExternalInput").ap()
    ctxT_d = nc.dram_tensor("ctxT", [D, M], BF16, kind="ExternalInput").ap()
    wq_d = nc.dram_tensor("wq", [D, D], BF16, kind="ExternalInput").ap()
    wkv_d = nc.dram_tensor("wkv", [D, 2 * D], BF16, kind="ExternalInput").ap()
    wo_d = nc.dram_tensor("wo", [D, D], BF16, kind="ExternalInput").ap()
    out_d = nc.dram_tensor("out", [RPC, D], F32, kind="ExternalOutput").ap()

    xT_r = xT_d.rearrange("(c p) n -> p c n", p=128)      # [128, 12, 1408]
    ctxT_r = ctxT_d.rearrange("(c p) n -> p c n", p=128)  # [128, 12, 512]
    wq_r = wq_d.rearrange("(c p) n -> p c n", p=128)
    wkv_r = wkv_d.rearrange("(c p) n -> p c n", p=128)
    wo_r = wo_d.rearrange("(c p) n -> p c n", p=128)

    with tile.TileContext(nc) as tc:
        with (
            tc.tile_pool(name="const", bufs=1) as constp,
            tc.tile_pool(name="wts", bufs=1) as wtp,
            tc.tile_pool(name="kv", bufs=1) as kvp,
            tc.tile_pool(name="io", bufs=2) as iop,
            tc.tile_pool(name="work", bufs=2) as workp,
            tc.tile_pool(name="ps", bufs=2, space="PSUM") as psp,
        ):
            # ---- constants ----
            ones_b = constp.tile([128, 1], BF16, name="ones_b")
            nc.vector.memset(ones_b[:], 1.0)
            epsb = constp.tile([1, 1], F32, name="epsb")
            nc.vector.memset(epsb[:], float(HD * EPS))

            wq_sb = wtp.tile([128, DC, D], BF16, name="wq_sb")
            wo_sb = wtp.tile([128, DC, D], BF16, name="wo_sb")

            kT_sb = kvp.tile([128, H, M], BF16, name="kT_sb")   # [dq, h, j]
            v_sb = kvp.tile([128, JB, D], BF16, name="v_sb")    # [j, jb, hd]
            ctxT_sb = kvp.tile([128, DC, M], BF16, name="ctxT_sb")

            nc.gpsimd.load_library(library_config.attn)

            # Pin the ACT spline-table set to the single set that holds
            # every function this kernel uses (Exp, Ln, Square). Without
            # this, the act-table-load pass alternates natural_log <->
            # exp_and_others around every Ln (2x ~1.3us loads per head).
            from concourse.hw_specs import get_activation_tables
            _tables = list(get_activation_tables(nc.m.arch))
            _set_id = _tables.index("natural_log_exp_and_others")
            nc.scalar.add_instruction(
                mybir.InstLoadActFuncSet(
                    name=f"I-{nc.next_id()}", ins=[], outs=[],
                    act_func_set_id=_set_id,
                ))

            def body():
                # DMA order matters: the kv projection's inputs (ctxT +
                # first wkv chunk) come first so the PE starts within ~10us;
                # wq/xg arrive during phase A's ~60us of kv matmuls.
                nc.sync.dma_start(out=ctxT_sb[:], in_=ctxT_r)

                # ---- phase A: kv projection (no transposes needed) ----
                for half, vc in ((0, 0), (1, 0), (0, 1), (1, 1), (0, 2),
                                 (1, 2)):
                    wch = workp.tile(
                        [128, DC, 512], BF16, name="wch", tag="big12k",
                        bufs=3)
                    nc.sync.dma_start(
                        out=wch[:],
                        in_=wkv_r[:, :, half * D + vc * 512:
                                  half * D + (vc + 1) * 512])
                    if half == 1 and vc == 0:
                        # emit mid-phase-A so these transfers don't starve
                        # the wkv chunk stream feeding the current matmuls
                        xgs = {}
                        xgs[0] = iop.tile([128, DC, 512], BF16,
                                          name="xg", tag="xg")
                        nc.sync.dma_start(out=xgs[0][:],
                                          in_=xT_r[:, :, 0:512])
                        nc.sync.dma_start(out=wq_sb[:], in_=wq_r)
                    if half == 0 and vc == 2:
                        nc.sync.dma_start(out=wo_sb[:], in_=wo_r)
                    if half == 0:
                        # kT_h = wkv_k_h^T @ ctxT : [dq 128, j 512]
                        for hh in range(4):
                            h = vc * 4 + hh
                            pk = psp.tile([128, 512], F32, name="pk",
                                          tag="qt", bufs=2)
                            for c in range(DC):
                                nc.tensor.matmul(
                                    pk[:],
                                    lhsT=wch[:, c, hh * 128:(hh + 1) * 128],
                                    rhs=ctxT_sb[:, c, :],
                                    start=(c == 0), stop=(c == DC - 1))
                            nc.vector.tensor_copy(kT_sb[:, h, :], pk[:])
                    else:
                        # v natural: [j 128, dv] per jb row-block
                        for jb in range(JB):
                            pv = psp.tile([128, 512], F32, name="pv",
                                          tag="sc", bufs=2)
                            for c in range(DC):
                                nc.tensor.matmul(
                                    pv[:],
                                    lhsT=ctxT_sb[:, c,
                                                 jb * 128:(jb + 1) * 128],
                                    rhs=wch[:, c, :],
                                    start=(c == 0), stop=(c == DC - 1))
                            nc.vector.tensor_copy(
                                v_sb[:, jb, vc * 512:(vc + 1) * 512],
                                pv[:])

                # ---- phase B: flat pipeline over (group, head) ----
                oTns = {}
                qTs, qsqs, qtns = {}, {}, {}

                def stage_qproj(gi, h):
                    g0, gn = GROUPS[gi]
                    gw = gn * 128
                    if h == 0:
                        oTns[gi] = workp.tile([128, H, 512], BF16,
                                              name="oTn", tag="big12k",
                                              bufs=3)
                        if gi + 1 < len(GROUPS):
                            ng0, ngn = GROUPS[gi + 1]
                            xgs[gi + 1] = iop.tile(
                                [128, DC, 512], BF16, name="xg", tag="xg")
                            nc.sync.dma_start(
                                out=xgs[gi + 1][:, :, :ngn * 128],
                                in_=xT_r[:, :,
                                         ng0 * 128:ng0 * 128 + ngn * 128])
                    qT = psp.tile([128, 512], F32, name="qT",
                                  tag="qt", bufs=2)
                    qTs[h % 2] = qT
                    for c in range(DC):
                        nc.tensor.matmul(
                            qT[:, :gw],
                            lhsT=wq_sb[:, c, h * 128:(h + 1) * 128],
                            rhs=xgs[gi][:, c, :gw],
                            start=(c == 0), stop=(c == DC - 1))
                    qsq = workp.tile([128, 512], BF16, name="qsq",
                                     tag="qsq", bufs=2)
                    qsqs[h % 2] = qsq
                    nc.scalar.activation(qsq[:, :gw], qT[:, :gw], SQUARE)

                def stage_rms(gi, h):
                    gw = GROUPS[gi][1] * 128
                    ssq = psp.tile([1, 512], F32, name="ssq",
                                   tag="ssq", bufs=1)
                    nc.tensor.matmul(
                        ssq[:, :gw], lhsT=ones_b[:],
                        rhs=qsqs[h % 2][:, :gw], start=True, stop=True)
                    sd = workp.tile([1, 512], F32, name="sd",
                                    tag="sd", bufs=2)
                    nc.scalar.activation(sd[:, :gw], ssq[:, :gw], LN,
                                         bias=epsb[:])
                    rs = workp.tile([1, 512], F32, name="rs",
                                    tag="rs", bufs=2)
                    nc.scalar.activation(rs[:, :gw], sd[:, :gw], EXP,
                                         scale=-0.5)
                    rsB = workp.tile([128, 512], F32, name="rsB",
                                     tag="rsB", bufs=2)
                    nc.gpsimd.partition_broadcast(rsB[:, :gw], rs[:, :gw])
                    qtn = workp.tile([128, 512], BF16, name="qtn",
                                     tag="qtn", bufs=3)
                    qtns[h % 3] = qtn
                    nc.vector.tensor_mul(
                        qtn[:, :gw], qTs[h % 2][:, :gw], rsB[:, :gw])

                def stage_attn(gi, h):
                    gw = GROUPS[gi][1] * 128
                    eT = workp.tile([128, JB, 512], BF16, name="eT",
                                    tag="eT", bufs=2)
                    for jb in range(JB):
                        sc = psp.tile([128, 512], F32, name="sc",
                                      tag="sc", bufs=2)
                        nc.tensor.matmul(
                            sc[:, :gw],
                            lhsT=kT_sb[:, h, jb * 128:(jb + 1) * 128],
                            rhs=qtns[h % 3][:, :gw], start=True, stop=True)
                        nc.scalar.activation(
                            eT[:, jb, :gw], sc[:, :gw], EXP)
                    sums = psp.tile([1, 512], F32, name="sums",
                                    tag="sums", bufs=1)
                    for jb in range(JB):
                        nc.tensor.matmul(
                            sums[:, :gw], lhsT=ones_b[:],
                            rhs=eT[:, jb, :gw],
                            start=(jb == 0), stop=(jb == JB - 1))
                    oTp = psp.tile([128, 512], F32, name="oTp",
                                   tag="ot", bufs=2)
                    for jb in range(JB):
                        nc.tensor.matmul(
                            oTp[:, :gw],
                            lhsT=v_sb[:, jb, h * 128:(h + 1) * 128],
                            rhs=eT[:, jb, :gw],
                            start=(jb == 0), stop=(jb == JB - 1))
                    rc = workp.tile([1, 512], F32, name="rc",
                                    tag="rc", bufs=2)
                    nc.vector.reciprocal_approx_fast(
                        rc[:, :gw], sums[:, :gw])
                    rcB = workp.tile([128, 512], F32, name="rcB",
                                     tag="rcB", bufs=2)
                    nc.gpsimd.partition_broadcast(rcB[:, :gw], rc[:, :gw])
                    nc.vector.tensor_mul(
                        oTns[gi][:, h, :gw], oTp[:, :gw], rcB[:, :gw])

                def outproj(gi):
                    g0, gn = GROUPS[gi]
                    for bi in range(gn):
                        ib = g0 + bi
                        for ec in range(3):
                            sl = slice(ec * 512, (ec + 1) * 512)
                            po = psp.tile([128, 512], F32, name="po",
                                          tag="sc", bufs=2)
                            for h in range(H):
                                nc.tensor.matmul(
                                    po[:],
                                    lhsT=oTns[gi][:, h,
                                                  bi * 128:(bi + 1) * 128],
                                    rhs=wo_sb[:, h, sl],
                                    start=(h == 0), stop=(h == H - 1))
                            och = workp.tile([128, 512], F32, name="och",
                                             tag="och", bufs=2)
                            nc.vector.tensor_copy(och[:], po[:])
                            nc.sync.dma_start(
                                out=out_d[ib * 128:(ib + 1) * 128, sl],
                                in_=och[:])

                # rms(h-1) is emitted BEFORE qproj(h): its ssq matmul only
                # needs Square(h-1) (already done on ACT), and starting the
                # Ln/Exp/bcast/mul chain ~2.6us earlier lets it free
                # qT(h-1) before qproj(h+1) needs the PSUM buffer.
                NSTEP = len(GROUPS) * H
                for step in range(NSTEP + 2):
                    if 1 <= step <= NSTEP:
                        stage_rms((step - 1) // H, (step - 1) % H)
                    if step < NSTEP:
                        stage_qproj(step // H, step % H)
                    if step >= 2:
                        a = step - 2
                        stage_attn(a // H, a % H)
                        if a % H == H - 1:
                            outproj(a // H)

            body()
    nc.finalize()
    return nc


def kernel(x, context, wq, bq, wkv, bkv, wo, bo, q_norm_scale):
    x = np.asarray(x, dtype=np.float32)
    context = np.asarray(context, dtype=np.float32)
    bf = ml_dtypes.bfloat16

    if "nc" not in _cache:
        _cache["nc"] = _build()
    nc = _cache["nc"]

    scale_t = np.tile(np.asarray(q_norm_scale, np.float32), H)      # [D]
    wkv_p = np.asarray(wkv, np.float32).copy()
    wkv_p[:, :D] *= scale_t[None, :]

    wq_b = np.asarray(wq, np.float32).astype(bf)
    wkv_b = wkv_p.astype(bf)
    wo_b = np.asarray(wo, np.float32).astype(bf)

    xp = np.zeros((B, CPB * RPC, D), np.float32)
    xp[:, :N] = x

    in_maps = []
    for core in range(NCORES):
        b, q = divmod(core, CPB)
        in_maps.append({
            "xT": np.ascontiguousarray(
                xp[b, q * RPC:(q + 1) * RPC].T).astype(bf),
            "ctxT": np.ascontiguousarray(context[b].T).astype(bf),
            "wq": wq_b, "wkv": wkv_b, "wo": wo_b,
        })

    res = bass_utils.run_bass_kernel_spmd(
        nc, in_maps, core_ids=list(range(NCORES)), trace=TRACE)
    _cache["last_results"] = res

    out = np.empty((B, N, D), np.float32)
    for b in range(B):
        cat = np.concatenate(
            [res.results[b * CPB + q]["out"] for q in range(CPB)], axis=0)
        out[b] = cat[:N]
    return out
